# revision 1
# baseline (speedup 1.0000x reference)
"""AcidSynth dry-path kernel for 8 Trainium2 NeuronCores (v4).

The reference output is `osc_gain * env * osc` with
  osc = (1-shape/2) * tanh(pi*partials*sin(arg)/2) * (1 + shape*cos(arg)),
  arg = f32(C * t), t = 1..N, C = f32(2*pi*f0/SR);
x / w_mod_sig / q_mod_sig are dead inputs (wet path discarded).

Sharding: sample-parallel, 524288 samples/core as [128 x 4096], split into
variable-width column chunks.  All engine work is generated on-device from
tiny per-(partition[,chunk]) constants.

Numerics (validated to rel_l2 ~3.3e-3 vs the reference; budget 2e-2):
  u  = f32(Ch*t), Ch = C/2 (exact halving), one tensor_scalar:
       (j + pbase_pc) * Ch  -- t = j+pbase exact int in f32, so u is
       bit-identical to the reference's half-angle phase (the f32
       quantization of C*t is what tanh amplifies at its transitions).
  Range reduction with a per-ROW pre-subtraction (Bf = K0*pi from the row
  start, split Bhi=f32(Bf) rounded down + Blo residual):
       s  = u - Bhi                        (exact: same-binade cancel)
       k8 = round_i32(s * invpi)           (RNE convert; k <= 6)
       tm = s - k8*f32(pi)                 (one stt; err k*(pi-f32(pi)) <
                                            6e-7 rad) = y + Blo,
       y in [-pi/2, pi/2] the centered half-angle.
  Blo rides the per-row ACT bias pointers:
  sq = Tanh(2D*tm - 2D*Blo) ~= tanh(D*sin(2y)): transitions at y=0
       coincide exactly; the mismatch near y=+-pi/2 is annihilated by
       cos^2(y).  D = pi*partials/2 ~ 576.
  cosv = Sin(-tm + (pi/2 + Blo)) = cos(y), arg in [0, pi] (table range).
  cp = cosv^2 (bf16); p = sq*cp (bf16 tensor_tensor, 2x mode)
  e2 = s2sq*(b2_pc - slope*j): envelope without intra-chunk wrap, with
       the osc scale s2sq folded into the host constants.
  outc = p * e2 -> bf16 DRAM (host upcasts to f32; bf16 rounding adds
       ~1e-3 rel_l2, far inside the tolerance and it halves the output
       DMA traffic).
Intra-chunk envelope wraps (env period L >= 6000 > max width, so <= 1
wrap per chunk) and the reference's linspace tail are recomputed exactly
on host over the affected samples (~4% of the stream).

Engine layout (per chunk): u/tm/cv2/p on DVE (tensor_scalar runs in 2x
mode even in f32; bf16 tensor_tensor 2x), s on GPSIMD (single-ALU
tensor_scalar is the only ptr-op shape Pool's ISA accepts -- multi-ALU
tensor_scalar/stt and PSUM reads are rejected by the BIR verifier),
k8 on ACT (Identity with int32 output = RNE convert), outc on GPSIMD
except the last chunks (DVE drains first).  The in-order sequencers make
emission order and the ~2.4us input-DMA latency part of the critical
path; the j row comes from a GPSIMD iota split so chunk 0's slice is
ready before the constants DMA lands, and small head/tail chunks
shorten pipeline fill/drain.

Schedule facts (measured via TimelineSim, do not re-derive): the fabric
floor (cst DMA + one DVE pass + 6 output DMAs + drain) is ~9.1us; the
ACT stream is the pipeline pacer (gap-free 13.1us) and ANY op inserted
into it -- even rate-favorable moves like e2-as-Identity or k8 swaps --
loses more to pacing than its static cost; DVE is the tail engine.
Grouping/staging/ramp emission orders, CW phase mixes, tail splits, and
~80 width/engine-map variants (incl. randomized joint sweeps) all lose
to this configuration.
"""
import numpy as np

import concourse.bass as bass
import concourse.mybir as mybir
import concourse.tile as tile
from concourse.bass_utils import run_bass_kernel_spmd

SR = 48000
MIN_MIDI, MAX_MIDI = 30, 60
MIN_DUR, MAX_DUR = 0.125, 0.5
N_SAMPLES = 4194304
N_CORES = 8
P = 128
FREE = 4096
S_CORE = P * FREE

DT = mybir.dt.float32
BF16 = mybir.dt.bfloat16
AFT = mybir.ActivationFunctionType
ALU = mybir.AluOpType

LAST_RESULTS = None

# engine maps are per-chunk strings of 'v' (DVE) or 'g' (GPSIMD); a single
# char applies to all chunks.  widths must sum to FREE.  sq_act is per-chunk
# 'a'/'d': Square on ACT vs cv2=cosv*cosv on the p-engine.
BEST_OPTS3 = dict(
    widths=(384, 704, 832, 832, 704, 640),
    u_eng='v',
    s_eng='g',
    k8_eng='a',
    tm_eng='v',
    phase='s',             # per-chunk: 's' Bhi-presub path, 'c' 2-term CW
    e2_eng='v',
    p_eng='v',
    outc_eng='ggggvv',
    sq_act='d',           # per ACT-GROUP when act_group>1
    act_group=1,          # chunks per ACT op group (int or tuple of sizes)
    e2_bf16=True,          # e2 as 4x-mode i16->bf16 tensor_scalar, s2sq folded
    out_bf16=True,         # DRAM out tensor in bf16; host upcasts to f32
    cosv_bf16=False,
    e2_pe=False,           # envelope via PE matmul into PSUM (exact x1.0)
    e2_pe_dma=False,       # bounce PSUM e2 to SBUF via DMA (GPSIMD can't
                           # read PSUM -- BIR verifier restriction)
    psum_bufs=4,
    j_dma=False,           # GPSIMD iota beats a DMA for the j row
    iota_split=True,       # chunk-0 j slice first (unblocks u0 sooner)
    u_first=(0, 1),        # head chunks: u before e2 (u feeds the pacer;
                           # for later chunks e2-first fills DVE better)
    stage_order=False,
    bufs=5,
    hoist_dmas=True,
)


def _percheck(val, n):
    s = str(val)
    if len(s) == 1:
        s = s * n
    assert len(s) == n, (val, n)
    return s


def _split_sync_waits(nc, max_waits=1, flip_engines=("Pool",)):
    """Walrus in this build rejects instructions carrying more than one sem
    wait (verified: 2 waits fail codegen with 'Too many sync wait
    commands').  Hoist extra waits onto same-engine NoOps inserted just
    before the offending instruction (same-engine streams execute in
    order; both waits are still enforced, so this is semantics-preserving).
    Which wait stays on the instruction is a free scheduling choice;
    keep-last wins overall except on Pool, where keep-first measures
    slightly better."""
    n = 0
    for f in nc.m.functions:
        for bb in f.blocks:
            insts = bb.instructions
            out = []
            for inst in insts:
                si = inst.sync_info
                if si is not None and si.on_wait and len(si.on_wait) > max_waits:
                    waits = list(si.on_wait)
                    flip = str(inst.engine).split('.')[-1] in flip_engines
                    keep = waits[:max_waits] if flip else waits[-max_waits:]
                    move = waits[max_waits:] if flip else waits[:-max_waits]
                    for w in move:
                        n += 1
                        nop = mybir.InstNoOp(
                            name=f"I-wsplit-{nc.next_id()}", ins=[], outs=[])
                        nop.engine = inst.engine
                        nop.sync_info = mybir.SyncInfo(on_wait=[w], on_update=[])
                        out.append(nop)
                    si.on_wait = keep
                out.append(inst)
            bb.instructions = out
    return n


def _hoist_input_dmas(nc, names=("cst", "cst2", "cst3")):
    """Move input-constant DMA triggers to the front of the entry block so
    they precede the Bass-init memsets/barrier (no dependency on preamble)."""
    f = nc.m.functions[0]
    blocks = list(f.blocks)
    hoisted = []
    for bb in blocks[1:]:
        insts = bb.instructions
        keep = []
        for inst in insts:
            is_target = False
            if "DMA" in type(inst).__name__ or \
               "dma" in (getattr(inst, "opcode", "") or "").lower():
                for arg in (inst.ins or []):
                    ref = getattr(arg, "memref", "") or ""
                    if any(ref == n or ref.startswith(n + "-") or
                           ref.startswith(n + "_") for n in names):
                        is_target = True
                        break
            (hoisted if is_target else keep).append(inst)
        if len(keep) != len(insts):
            bb.instructions = keep
    if hoisted:
        bb0 = blocks[0]
        insts0 = bb0.instructions
        cut = 1 if insts0 and type(insts0[0]).__name__ == "InstCall" else 0
        bb0.instructions = insts0[:cut] + hoisted + insts0[cut:]
    return len(hoisted)


def _build3(consts, opts=None):
    o = dict(BEST_OPTS3)
    if opts:
        o.update(opts)
    widths = list(o["widths"])
    n_chunks = len(widths)
    assert sum(widths) == FREE, widths
    w_max = max(widths)
    offs = np.concatenate([[0], np.cumsum(widths)[:-1]]).astype(int)
    ag = o["act_group"]
    if isinstance(ag, (list, tuple)):
        groups, i = [], 0
        for g in ag:
            groups.append(list(range(i, min(i + g, n_chunks))))
            i += g
        assert i == n_chunks, (ag, n_chunks)
    else:
        groups = [list(range(g, min(g + ag, n_chunks)))
                  for g in range(0, n_chunks, ag)]
    n_groups = len(groups)
    u_eng = _percheck(o["u_eng"], n_chunks)
    s_eng = _percheck(o.get("s_eng", 'v'), n_chunks)
    k8_eng = _percheck(o.get("k8_eng", 'v'), n_chunks)
    tm_eng = _percheck(o["tm_eng"], n_chunks)
    phase = _percheck(o.get("phase", 's'), n_chunks)
    e2_eng = _percheck(o["e2_eng"], n_chunks)
    p_eng = _percheck(o["p_eng"], n_chunks)
    outc_eng = _percheck(o["outc_eng"], n_chunks)
    sq_act = _percheck(o["sq_act"], n_groups)

    Ch = float(consts["Ch"])
    PI32 = float(np.float32(np.pi))
    INVPI = float(np.float32(1.0 / np.pi))
    HALFPI = float(np.float32(np.pi / 2))
    P1 = 3.140625
    P2 = float(np.float32(np.pi - P1))
    twoD = float(consts["twoD"])
    s2 = float(consts["s2"])
    s2sq = float(consts["s2sq"])
    slope = float(consts["slope"])

    nc = bass.Bass("TRN2", target_bir_lowering=False)
    # cst cols: [pbase x n | Bhi | bias_t | bias_s | zero | halfpi | b2 x n]
    ncst = 2 * n_chunks + 5
    cst = nc.dram_tensor("cst", [P, ncst], DT, kind="ExternalInput")
    if o["j_dma"]:
        cst3 = nc.dram_tensor("cst3", [P, w_max], mybir.dt.int16,
                              kind="ExternalInput")
    if o["e2_pe"]:
        # cst2[:, c*128+p] = (b2_pc, 1); cst2[:, n_chunks*128+n] = (1, -slope*n)
        cst2 = nc.dram_tensor("cst2", [2, n_chunks * P + w_max], DT,
                              kind="ExternalInput")
    out_dt = BF16 if o["out_bf16"] else DT
    out = nc.dram_tensor("out", [P, FREE], out_dt, kind="ExternalOutput")

    def eng(c):
        return {"v": nc.vector, "g": nc.gpsimd}[c]

    with tile.TileContext(nc) as tc:
        with (
            tc.tile_pool(name="glob", bufs=1) as glob,
            tc.tile_pool(name="work", bufs=1) as work,
            tc.tile_pool(name="psum", bufs=1, space="PSUM") as psum,
        ):
            cst_t = glob.tile([P, ncst], DT, name="cst_t", tag="cst_t")
            nc.sync.dma_start(cst_t[:], cst[:])
            jt = glob.tile([P, w_max], mybir.dt.int16, name="jt", tag="jt")
            if o["j_dma"]:
                nc.sync.dma_start(jt[:], cst3[:])
            elif o.get("iota_split") and widths[0] < w_max:
                # chunk-0 slice first so u0 isn't gated by the full iota
                w0 = widths[0]
                nc.gpsimd.iota(jt[:, 0:w0], pattern=[[1, w0]], base=0,
                               channel_multiplier=0)
                nc.gpsimd.iota(jt[:, w0:w_max], pattern=[[1, w_max - w0]],
                               base=w0, channel_multiplier=0)
            else:
                nc.gpsimd.iota(jt[:], pattern=[[1, w_max]], base=0,
                               channel_multiplier=0)
            jt = jt[:, 0:w_max]
            if o["e2_pe"]:
                cst2_t = glob.tile([2, n_chunks * P + w_max], DT,
                                   name="cst2_t", tag="cst2_t")
                nc.sync.dma_start(cst2_t[:], cst2[:])
                rhs_env = cst2_t[:, n_chunks * P:]
            # touch the ACT table before the loop (amortized table load)
            dummy = glob.tile([P, 1], DT, name="dummy", tag="dummy")
            nc.scalar.activation(dummy[:], cst_t[:, 0:1], AFT.Sin, scale=0.0)

            sc = 0
            bhi_ap = cst_t[:, sc + n_chunks:sc + n_chunks + 1]
            bias_t_ap = cst_t[:, sc + n_chunks + 1:sc + n_chunks + 2]
            bias_s_ap = cst_t[:, sc + n_chunks + 2:sc + n_chunks + 3]
            zero_ap = cst_t[:, sc + n_chunks + 3:sc + n_chunks + 4]
            halfpi_ap = cst_t[:, sc + n_chunks + 4:sc + n_chunks + 5]
            us, tmg, sqg, cosvg, cpg, ps, e2s = ({} for _ in range(7))
            goff = {}   # chunk -> (group_idx, col offset within group tile)
            gw = {}
            for gi, grp in enumerate(groups):
                so = 0
                for c in grp:
                    goff[c] = (gi, so)
                    so += widths[c]
                gw[gi] = so

            def em_u(c):
                w = widths[c]
                pbase = cst_t[:, sc + c:sc + c + 1]
                u = work.tile([P, w], DT, name=f"u{c}", tag=f"u{c}", bufs=1)
                eng(u_eng[c]).tensor_scalar(
                    u[:], jt[:, 0:w], pbase, Ch, ALU.add, ALU.mult)
                us[c] = u

            sks = {}
            # k8_pair: pairs of adjacent chunks sharing one s tile and ONE
            # ACT k8 op (fewer pacer-stream ops); consumers read slices.
            k8_pairs = tuple(tuple(p) for p in (o.get("k8_pair") or ()))
            pair_of = {}
            for a, b in k8_pairs:
                assert b == a + 1, (a, b)
                pair_of[a] = (a, b)
                pair_of[b] = (a, b)
            pair_tiles = {}

            def em_sk(c):
                w = widths[c]
                if c in pair_of:
                    a, b = pair_of[c]
                    wa, wb = widths[a], widths[b]
                    if (a, b) not in pair_tiles:
                        sp = work.tile([P, wa + wb], DT, name=f"sp{a}",
                                       tag=f"sp{a}", bufs=1)
                        kp = work.tile([P, wa + wb], mybir.dt.int32,
                                       name=f"kp{a}", tag=f"kp{a}", bufs=1)
                        pair_tiles[(a, b)] = (sp, kp)
                    sp, kp = pair_tiles[(a, b)]
                    lo = 0 if c == a else wa
                    eng(s_eng[c]).tensor_scalar_sub(
                        sp[:, lo:lo + w], us[c][:], bhi_ap)
                    sks[c] = (sp[:, lo:lo + w], kp[:, lo:lo + w])
                    if c == b:
                        # both slices written: one grouped ACT convert
                        nc.scalar.activation(kp[:], sp[:], AFT.Identity,
                                             bias=0.0, scale=INVPI)
                    return sks[c]
                s = work.tile([P, w], DT, name=f"s{c}", tag=f"s{c}", bufs=1)
                eng(s_eng[c]).tensor_scalar_sub(s[:], us[c][:], bhi_ap)
                k8 = work.tile([P, w], mybir.dt.int32, name=f"k{c}",
                               tag=f"k{c}", bufs=1)
                if k8_eng[c] == 'a':
                    nc.scalar.activation(k8[:], s[:], AFT.Identity,
                                         bias=0.0, scale=INVPI)
                else:
                    eng(k8_eng[c]).tensor_scalar_mul(k8[:], s[:], INVPI)
                sks[c] = (s, k8)
                return s, k8

            def em_tm(c):
                w = widths[c]
                gi, so = goff[c]
                if gi not in tmg:
                    tmg[gi] = work.tile([P, gw[gi]], DT, name=f"tmg{gi}",
                                        tag=f"tmg{gi}", bufs=1)
                if phase[c] == 'c':
                    # 2-term Cody-Waite from u: k*P1 exact (k < 2^15)
                    k8 = work.tile([P, w], mybir.dt.int32, name=f"k{c}",
                                   tag=f"k{c}", bufs=1)
                    if k8_eng[c] == 'a':
                        nc.scalar.activation(k8[:], us[c][:], AFT.Identity,
                                             bias=0.0, scale=INVPI)
                    else:
                        eng(k8_eng[c]).tensor_scalar_mul(
                            k8[:], us[c][:], INVPI)
                    r1 = work.tile([P, w], DT, name=f"r1{c}", tag=f"r1{c}",
                                   bufs=1)
                    eng(tm_eng[c]).scalar_tensor_tensor(
                        r1[:], k8[:], -P1, us[c][:], ALU.mult, ALU.add)
                    eng(tm_eng[c]).scalar_tensor_tensor(
                        tmg[gi][:, so:so + w], k8[:], -P2, r1[:],
                        ALU.mult, ALU.add)
                    return
                if c in sks:
                    s, k8 = sks[c]
                else:
                    s, k8 = em_sk(c)
                s_ap = s if isinstance(s, bass.AP) else s[:]
                k8_ap = k8 if isinstance(k8, bass.AP) else k8[:]
                eng(tm_eng[c]).scalar_tensor_tensor(
                    tmg[gi][:, so:so + w], k8_ap, -PI32, s_ap,
                    ALU.mult, ALU.add)

            def em_act(gi):
                g = gw[gi]
                ph = phase[groups[gi][0]]
                assert all(phase[c] == ph for c in groups[gi]), "mixed group"
                bt_ap = bias_t_ap if ph == 's' else zero_ap
                bs_ap = bias_s_ap if ph == 's' else halfpi_ap
                on_act = sq_act[gi] == 'a'
                cosv_dt = DT if (on_act and not o["cosv_bf16"]) else BF16
                cosv = work.tile([P, g], cosv_dt, name=f"cosv{gi}",
                                 tag=f"cosv{gi}", bufs=1)
                nc.scalar.activation(cosv[:], tmg[gi][:], AFT.Sin,
                                     bias=bs_ap, scale=-1.0)
                cosvg[gi] = cosv
                sq = work.tile([P, g], BF16, name=f"sq{gi}", tag=f"sq{gi}",
                               bufs=1)
                nc.scalar.activation(sq[:], tmg[gi][:], AFT.Tanh,
                                     bias=bt_ap, scale=twoD)
                sqg[gi] = sq
                if on_act:
                    cp = work.tile([P, g], BF16, name=f"cp{gi}", tag=f"cp{gi}",
                                   bufs=1)
                    sc_sq = 1.0 if o["e2_bf16"] else s2
                    nc.scalar.activation(cp[:], cosv[:], AFT.Square,
                                         scale=sc_sq)
                    cpg[gi] = cp

            def em_prod(c, lo=0, hi=None):
                w = widths[c]
                hi = w if hi is None else hi
                gi, so = goff[c]
                if sq_act[gi] != 'a':
                    if gi not in cpg:
                        cpg[gi] = work.tile([P, gw[gi]], BF16, name=f"cp{gi}",
                                            tag=f"cp{gi}", bufs=1)
                    cv = cosvg[gi][:, so + lo:so + hi]
                    eng(p_eng[c]).tensor_tensor(
                        cpg[gi][:, so + lo:so + hi], cv, cv, ALU.mult)
                if c not in ps:
                    ps[c] = work.tile([P, w], BF16, name=f"p{c}", tag=f"p{c}",
                                      bufs=1)
                eng(p_eng[c]).tensor_tensor(
                    ps[c][:, lo:hi], sqg[gi][:, so + lo:so + hi],
                    cpg[gi][:, so + lo:so + hi], ALU.mult)

            def em_e2(c):
                w = widths[c]
                if o["e2_pe"]:
                    e2 = psum.tile([P, w_max], DT, name=f"e2p{c}", tag="e2p",
                                   bufs=o["psum_bufs"])
                    e2 = e2[:, 0:w]
                    # matmul per <=512-col slice: PSUM writes stay in-bank
                    for s0 in range(0, w, 512):
                        s1 = min(s0 + 512, w)
                        nc.tensor.matmul(
                            e2[:, s0:s1], cst2_t[:, c * P:(c + 1) * P],
                            rhs_env[:, s0:s1], start=True, stop=True)
                    if o["e2_pe_dma"]:
                        e2s_sb = work.tile([P, w], DT, name=f"e2s{c}",
                                           tag=f"e2s{c}", bufs=1)
                        nc.sync.dma_start(e2s_sb[:], e2)
                        e2 = e2s_sb[:]
                else:
                    b2 = cst_t[:, sc + n_chunks + 5 + c:sc + n_chunks + 6 + c]
                    e2_dt = BF16 if o["e2_bf16"] else DT
                    e2 = work.tile([P, w], e2_dt, name=f"e2{c}", tag=f"e2{c}",
                                   bufs=1)
                    sl = -slope * s2sq if o["e2_bf16"] else -slope
                    ka = int(dict(o.get("e2_act_cols") or {}).get(c, 0))
                    if 0 < ka <= w:
                        # first ka columns on ACT (Identity, bias ptr): e2
                        # has no bit-compat constraint, and ACT's 1.6x rate
                        # premium is the cheapest relief for the DVE max
                        nc.scalar.activation(e2[:, 0:ka], jt[:, 0:ka],
                                             AFT.Identity, bias=b2, scale=sl)
                        if ka < w:
                            eng(e2_eng[c]).tensor_scalar(
                                e2[:, ka:w], jt[:, ka:w], sl, b2,
                                ALU.mult, ALU.add)
                    else:
                        eng(e2_eng[c]).tensor_scalar(
                            e2[:], jt[:, 0:w], sl, b2, ALU.mult, ALU.add)
                    e2 = e2[:]
                e2s[c] = e2

            outcs = {}

            def em_outc(c, lo=0, hi=None):
                w = widths[c]
                hi0 = hi
                hi = w if hi is None else hi
                gi, _ = goff[c]
                outc_dt = BF16 if o["out_bf16"] else DT
                if c not in outcs:
                    outcs[c] = work.tile([P, w], outc_dt, name=f"outc{c}",
                                         tag=f"outc{c}", bufs=1)
                outc_f = outcs[c]
                if hi0 is not None or lo:
                    eng(outc_eng[c]).tensor_tensor(
                        outc_f[:, lo:hi], ps[c][:, lo:hi], e2s[c][:, lo:hi],
                        ALU.mult)
                    nc.sync.dma_start(
                        out[:, offs[c] + lo:offs[c] + hi], outc_f[:, lo:hi])
                    return
                outc = outc_f
                spl = dict(o.get("outc_split") or {}).get(c, 0)
                if o["e2_bf16"]:
                    # s2sq folded into e2; all-bf16 tensor_tensor (2x mode)
                    if 0 < spl < w:
                        # first spl columns on the primary engine, rest on
                        # the other -- fine-grained V/G load balance
                        other = 'v' if outc_eng[c] == 'g' else 'g'
                        eng(outc_eng[c]).tensor_tensor(
                            outc[:, 0:spl], ps[c][:, 0:spl],
                            e2s[c][:, 0:spl], ALU.mult)
                        eng(other).tensor_tensor(
                            outc[:, spl:w], ps[c][:, spl:w],
                            e2s[c][:, spl:w], ALU.mult)
                    else:
                        eng(outc_eng[c]).tensor_tensor(
                            outc[:], ps[c][:], e2s[c], ALU.mult)
                else:
                    out_scalar = 1.0 if sq_act[gi] == 'a' else s2sq
                    eng(outc_eng[c]).scalar_tensor_tensor(
                        outc[:], ps[c][:], out_scalar, e2s[c],
                        ALU.mult, ALU.mult)
                nc.sync.dma_start(
                    out[:, offs[c]:offs[c] + w], outc[:])

            if o["stage_order"]:
                for c in range(n_chunks):
                    em_e2(c)
                for c in range(n_chunks):
                    em_u(c)
                    em_tm(c)
                for gi in range(n_groups):
                    em_act(gi)
                for c in range(n_chunks):
                    em_prod(c)
                for c in range(n_chunks):
                    em_outc(c)
            elif o.get("head_stage"):
                ahead = int(o.get("head_ahead", n_chunks))
                emitted = 0
                for gi, grp in enumerate(groups):
                    for c in grp:
                        while emitted < min(n_chunks, max(c + 1, ahead)):
                            cc = emitted
                            em_e2(cc)
                            em_u(cc)
                            em_sk(cc)
                            emitted += 1
                        em_tm(c)
                    em_act(gi)
                    for c in grp:
                        em_prod(c)
                        em_outc(c)
            else:
                tsplit = dict(o.get("tail_split") or {})
                e2_late = o.get("e2_late", False)
                uf = o.get("u_first", False)
                uf_set = set(range(n_chunks)) if uf is True else set(uf or ())
                for gi, grp in enumerate(groups):
                    for c in grp:
                        if c in uf_set:
                            em_u(c)
                        if not e2_late:
                            em_e2(c)
                        if c not in uf_set:
                            em_u(c)
                        em_tm(c)
                    em_act(gi)
                    for c in grp:
                        if e2_late:
                            em_e2(c)
                        k = int(tsplit.get(c, 0))
                        if 0 < k < widths[c]:
                            em_prod(c, 0, k)
                            em_outc(c, 0, k)
                            em_prod(c, k, None)
                            em_outc(c, k, None)
                        else:
                            em_prod(c)
                            em_outc(c)
    return nc


def _prepare3(inputs, opts=None):
    """Host scalar math + per-core constant tensors."""
    o = dict(BEST_OPTS3)
    if opts:
        o.update(opts)
    widths = list(o["widths"])
    n_chunks = len(widths)
    w_max = max(widths)
    offs = np.concatenate([[0], np.cumsum(widths)[:-1]]).astype(int)

    nod = float(np.asarray(inputs["note_on_duration_0to1"]).reshape(-1)[0])
    dur = nod * (MAX_DUR - MIN_DUR) + MIN_DUR
    L = int(dur * SR)
    slope = 1.0 / (L - 1)
    slope32 = np.float32(slope)

    midi = round(float(np.asarray(inputs["midi_f0_0to1"]).reshape(-1)[0])
                 * (MAX_MIDI - MIN_MIDI) + MIN_MIDI)
    f0_hz = 440.0 * 2.0 ** ((midi - 69) / 12.0)
    C = np.float32(2.0 * np.pi * f0_hz / SR)
    partials32 = np.float32(SR / (2.0 * f0_hz))
    B = np.float32(np.pi * float(partials32))
    D = np.float32(B / np.float32(2.0))

    shape32 = np.float32(np.asarray(inputs["osc_shape"]).reshape(-1)[0])
    gain32 = np.float32(np.asarray(inputs["osc_gain"]).reshape(-1)[0])
    g1_32 = np.float32(1.0) - shape32 / np.float32(2.0)
    qg2 = 2.0 * float(gain32) * float(g1_32)
    s2 = float(np.float32(np.sqrt(qg2 * float(shape32))))

    Ch = float(C) / 2.0            # exact power-of-two halving of f32 C
    D64 = float(D)
    consts = dict(L=L, slope=float(slope32), Ch=Ch, twoD=float(2.0 * D64),
                  s2=s2, s2sq=float(np.float32(s2) * np.float32(s2)))

    pe_env = bool(o.get("e2_pe"))
    j_dma = bool(o.get("j_dma"))
    e2_bf16 = bool(o.get("e2_bf16"))
    jrow = np.tile(np.arange(w_max, dtype=np.int16), (P, 1))
    rhs = np.zeros((2, w_max), np.float32)
    rhs[0, :] = 1.0
    rhs[1, :] = (np.arange(w_max, dtype=np.float64)
                 * (-np.float64(slope32))).astype(np.float32)
    in_maps = []
    for c in range(N_CORES):
        base = c * S_CORE + np.arange(P, dtype=np.int64) * FREE
        cst = np.zeros((P, 2 * n_chunks + 5), np.float32)
        cst2 = np.zeros((2, n_chunks * P + w_max), np.float32)
        # per-ROW phase base: K0 from the row start, valid across all 4096
        # cols (residual quotient <= ~8 after the mod)
        umin = Ch * (base + 1).astype(np.float64)
        K0 = np.floor(umin / np.pi).astype(np.int64)
        Bf = K0.astype(np.float64) * np.pi
        Bhi = Bf.astype(np.float32)
        over = Bhi.astype(np.float64) > Bf
        Bhi[over] = np.nextafter(Bhi[over], np.float32(-np.inf))
        Blo = Bf - Bhi.astype(np.float64)            # >= 0, < ulp
        sc = 0
        cst[:, sc + n_chunks] = Bhi
        cst[:, sc + n_chunks + 1] = (-2.0 * D64 * Blo).astype(np.float32)
        cst[:, sc + n_chunks + 2] = (np.pi / 2 + Blo).astype(np.float32)
        cst[:, sc + n_chunks + 3] = 0.0
        cst[:, sc + n_chunks + 4] = np.float32(np.pi / 2)
        s2sq32 = np.float32(consts["s2sq"])
        for ch in range(n_chunks):
            t0 = base + int(offs[ch])
            cst[:, sc + ch] = (t0 + 1).astype(np.float32)     # pbase
            m0 = t0 % L
            b2 = np.float32(1.0) - m0.astype(np.float32) * slope32
            cst[:, sc + n_chunks + 5 + ch] = b2 * s2sq32 if e2_bf16 else b2
            cst2[0, ch * P:(ch + 1) * P] = b2
            cst2[1, ch * P:(ch + 1) * P] = 1.0
        cst2[:, n_chunks * P:] = rhs
        m = {"cst": cst}
        if pe_env:
            m["cst2"] = cst2
        if j_dma:
            m["cst3"] = jrow
        in_maps.append(m)

    host = dict(L=L, slope=slope, slope32=slope32, C=C, B=B,
                shape32=shape32, gain32=gain32,
                widths=widths, offs=offs)
    return consts, in_maps, host


def _host_fixup(full, host):
    """Recompute, in float64 on host, the samples whose on-device envelope
    was wrong: intra-chunk envelope wraps and the reference's linspace
    tail.  Mirrors the reference bit-for-bit (f32 arg, f64 trig)."""
    n = full.shape[0]
    L = host["L"]
    slope32 = host["slope32"]
    C, B = host["C"], host["B"]
    shape32, gain32 = host["shape32"], host["gain32"]
    widths = np.asarray(host["widths"], dtype=np.int64)
    offs = np.asarray(host["offs"], dtype=np.int64)
    r_tail = n % L

    idx = np.arange(n, dtype=np.int64)
    col = idx % FREE
    chunk = np.searchsorted(offs, col, side="right") - 1
    g0 = idx - (col - offs[chunk])           # chunk-start sample index
    m = idx % L
    m0c = g0 % L
    fix = m < m0c                            # wrap occurred inside the chunk
    if r_tail > 0:
        fix |= idx >= (n - r_tail)
    ii = idx[fix]
    if ii.size == 0:
        return full

    t32 = (ii + 1).astype(np.float32)
    arg = (np.float32(C) * t32).astype(np.float32)
    a64 = arg.astype(np.float64)
    sq = np.tanh(float(B) * np.sin(a64) / 2.0)
    osc = (1.0 - float(shape32) / 2.0) * sq * (1.0 + float(shape32) * np.cos(a64))
    env = 1.0 - (ii % L).astype(np.float64) * np.float64(slope32)
    if r_tail > 0:
        end_val = max(1.0 - r_tail * float(slope32), 0.0)
        lin = np.linspace(1.0, end_val, r_tail, dtype=np.float32)
        tt = ii >= (n - r_tail)
        env[tt] = lin[(ii[tt] - (n - r_tail))]
    full[ii] = (float(gain32) * env * osc).astype(np.float32)
    return full


def kernel(**inputs) -> np.ndarray:
    global LAST_RESULTS
    x = np.asarray(inputs["x"])
    n = x.shape[-1]
    assert n == N_SAMPLES, f"kernel hardcoded for {N_SAMPLES}, got {n}"

    consts, in_maps, host = _prepare3(inputs)
    nc = _build3(consts, BEST_OPTS3)
    if BEST_OPTS3.get("hoist_dmas"):
        _hoist_input_dmas(nc)
    _split_sync_waits(nc)
    res = run_bass_kernel_spmd(nc, in_maps, core_ids=list(range(N_CORES)))
    LAST_RESULTS = res

    full = np.concatenate([np.asarray(res.results[c]["out"]).reshape(-1)
                           for c in range(N_CORES)]).astype(np.float32)
    full = _host_fixup(full, host)
    return full.reshape(1, n).astype(np.float32, copy=False)

